# revision 10
# baseline (speedup 1.0000x reference)
"""GCN (2-layer, PyG GCNConv semantics) on 8 Trainium2 NeuronCores.

Strategy (v3)
-------------
Host does layout + O(N) normalization; each NeuronCore does only dense
row-sum reductions over degree-class-padded bf16 message grids.

  layer-1 grid slot for edge u->v : x'(u)  = dinv(u)*x(u)   (+ self slot)
  layer-2 grids                   : y1'(u) = dinv(u)*y1(u)  (+ self slot)

With C_in == 1 and b1 == 0, layer 2 needs only s+(v) = sum of positive
incoming messages and s-(v) = sum of negative ones:
  out[v,j] = dinv_v*(a_j*s+ + c_j*s-) + b2_j,
  a_j = sum_{W1c>0} W1c*W2[c,j],  c_j = sum_{W1c<0} W1c*W2[c,j].
The host *sign-splits* the layer-2 messages into a positives grid and a
negatives grid (same total slot count!), so the device computes s+/s- as
plain row-sums - no abs/relu pass, no second reduction sweep.

Both NEFFs are instances of one generic "chunked grid row-sum" kernel:
DMA a column chunk, then per degree-class piece do an optional bf16
pairwise-add halving (DVE 2x mode) followed by a 1x tensor_reduce.
Node->row packing, unpacking, y1' = dinv^2*sums, and the 4-wide output
combine are all O(N)/O(E) index work + O(N) flops on the host.

Nodes are sorted by row width and dealt round-robin to the 8 cores
(independently per grid), so all cores share one SPMD NEFF geometry and
work is balanced to <0.1%; no collectives (a node's in-edges live on one
core).
"""
import sys

sys.path.insert(0, "/opt/trn_rl_repo")

import numpy as np

N_CORES = 8
CLS_STEP = 4          # slot-count class granularity
CHUNK_COLS = 2304     # <=4.6KB/partition bf16 per DMA chunk
HALVE_MIN_COLS = 768  # min piece cols to add the 2x pairwise-add pass

_NEFF_CACHE: dict = {}


class _Section:
    """Packing of one grid section (one slot-count distribution)."""

    def __init__(self, slot_counts):
        """slot_counts: [N] ints (0 => node absent from this section)."""
        n = slot_counts.shape[0]
        active = np.flatnonzero(slot_counts > 0)
        sc = slot_counts[active]
        order = np.argsort(sc, kind="stable")
        nodes = active[order]                   # width-sorted node ids
        widths = sc[order]
        clsS = (CLS_STEP * np.ceil(widths / CLS_STEP)).astype(np.int64)
        Svals, starts, cnts = np.unique(clsS, return_index=True,
                                        return_counts=True)
        self.classes = []                       # [(S, rpp)]
        for S, cnt in zip(Svals, cnts):
            npc = -(-int(cnt) // N_CORES)
            self.classes.append((int(S), -(-npc // 128)))
        # per active node: core, partition, row, class S
        i = np.arange(nodes.shape[0], dtype=np.int64)
        ci = np.searchsorted(Svals, clsS)
        rpp_arr = np.array([r for _, r in self.classes], np.int64)
        S_arr = Svals
        core = i % N_CORES
        q = (i - starts[ci]) // N_CORES
        p = q // rpp_arr[ci]
        r = q % rpp_arr[ci]
        goff = np.zeros(len(Svals), np.int64)
        ooff = np.zeros(len(Svals), np.int64)
        go = oo = 0
        for k, (S, rpp) in enumerate(self.classes):
            goff[k] = go
            ooff[k] = oo
            go += S * rpp
            oo += rpp
        self.gcols = go
        self.rpt = oo
        # per-active-node placement; caller adds section bases + core
        # stride to form flat indices.
        self.nodes = nodes
        self.core = core
        self.gbase = goff[ci] + r * S_arr[ci]
        self.obase = ooff[ci] + r
        self.p = p


def _plan_pieces(sections):
    """Lay out sections side by side in one [128, GCOLS] grid with one
    [128, RPT] sums output; return (GCOLS, RPT, chunks)."""
    pieces = []
    gbase = obase = 0
    for sec in sections:
        go = oo = 0
        for S, rpp in sec.classes:
            max_rows = max(1, CHUNK_COLS // S)
            r = 0
            while r < rpp:
                rows = min(max_rows, rpp - r)
                pieces.append((S, gbase + go + r * S, rows,
                               obase + oo + r))
                r += rows
            go += S * rpp
            oo += rpp
        gbase += sec.gcols
        obase += sec.rpt
    GCOLS, RPT = gbase, obase

    # pack pieces (grid-contiguous) into DMA chunks
    chunks = []
    cur, cur_cols = [], 0
    for (S, g0, rows, o0) in pieces:
        cols = rows * S
        if cur and cur_cols + cols > CHUNK_COLS + CHUNK_COLS // 2:
            chunks.append(cur)
            cur, cur_cols = [], 0
        cur.append((S, g0, g0 + cols, o0, o0 + rows))
        cur_cols += cols
    if cur:
        chunks.append(cur)
    out = [(ch[0][1], ch[-1][2], ch) for ch in chunks]
    # ascending size: a small first chunk starts the DVE quickly and the
    # DMA stream stays just ahead of the reduce stream
    out.sort(key=lambda c: c[1] - c[0])
    return GCOLS, RPT, out


def _build_neff(geom):
    """Generic chunked row-sum kernel: sums[:, o] = rowsum(g[:, piece])."""
    from concourse import bacc, mybir, tile

    GCOLS, RPT, chunks = geom
    nc = bacc.Bacc("TRN2", target_bir_lowering=False, debug=False,
                   num_devices=N_CORES, enable_partition_id=False)
    f32, bf16 = mybir.dt.float32, mybir.dt.bfloat16
    add = mybir.AluOpType.add
    X = mybir.AxisListType.X
    g = nc.dram_tensor("g", [128, GCOLS], bf16, kind="ExternalInput")
    sm = nc.dram_tensor("sm", [128, RPT], f32, kind="ExternalOutput")

    with tile.TileContext(nc) as tc:
        with tc.tile_pool(name="p", bufs=3) as pool, \
             tc.tile_pool(name="h", bufs=2) as hpool, \
             tc.tile_pool(name="s", bufs=1) as spool:
            sums = spool.tile([128, RPT], f32)
            for (g0, g1, pcs) in chunks:
                t = pool.tile([128, g1 - g0], bf16, tag="g")
                nc.sync.dma_start(out=t[:], in_=g.ap()[:, g0:g1])
                for (S, ig0, ig1, o0, o1) in pcs:
                    t3 = t[:, ig0 - g0:ig1 - g0].rearrange(
                        "p (r s) -> p r s", s=S)
                    if ig1 - ig0 >= HALVE_MIN_COLS and S % 2 == 0:
                        S2 = S // 2
                        h = hpool.tile([128, (o1 - o0) * S2], bf16,
                                       tag="h")
                        h3 = h[:].rearrange("p (r s) -> p r s", s=S2)
                        nc.vector.tensor_tensor(
                            out=h3, in0=t3[:, :, 0:S2],
                            in1=t3[:, :, S2:S], op=add)
                        nc.vector.tensor_reduce(
                            out=sums[:, o0:o1], in_=h3, axis=X, op=add)
                    else:
                        nc.vector.tensor_reduce(
                            out=sums[:, o0:o1], in_=t3, axis=X, op=add)
            nc.sync.dma_start(out=sm.ap(), in_=sums[:])
    nc.compile()
    return nc


def _get_neff(geom_key, geom):
    if geom_key not in _NEFF_CACHE:
        _NEFF_CACHE[geom_key] = _build_neff(geom)
    return _NEFF_CACHE[geom_key]


def _geom_key(geom):
    GCOLS, RPT, chunks = geom
    return (GCOLS, RPT,
            tuple((g0, g1, tuple(pcs)) for g0, g1, pcs in chunks))


def _run(geom, grids):
    """grids: [N_CORES, 128, GCOLS] bf16 -> sums [N_CORES, 128, RPT]."""
    from concourse import bass_utils

    nc = _get_neff(_geom_key(geom), geom)
    in_maps = [{"g": grids[c]} for c in range(N_CORES)]
    res = bass_utils.run_bass_kernel_spmd(nc, in_maps,
                                          core_ids=list(range(N_CORES)))
    return np.stack([res.results[c]["sm"] for c in range(N_CORES)])


def kernel(x, edge_index, W1, b1, W2, b2):
    from ml_dtypes import bfloat16

    x = np.asarray(x, dtype=np.float32)
    W1 = np.asarray(W1, dtype=np.float32).reshape(-1)   # [4] (C_in == 1)
    b1 = np.asarray(b1, dtype=np.float32).reshape(-1)
    W2 = np.asarray(W2, dtype=np.float32)               # [4, 4]
    b2 = np.asarray(b2, dtype=np.float32).reshape(-1)
    ei = np.asarray(edge_index)
    N = x.shape[0]
    E = ei.shape[1]
    assert x.shape[1] == 1 and W1.shape[0] == 4 and W2.shape == (4, 4)
    # b1 == 0 is load-bearing for the s+/s- collapse (spec: fill zeros).
    assert np.all(b1 == 0.0), "kernel specialized to b1 == 0"

    src = ei[0].astype(np.int64)
    dst = ei[1].astype(np.int64)

    # ---- shared host index work ----
    indeg = np.bincount(dst, minlength=N).astype(np.int64)
    slots = indeg + 1                                   # + self slot
    dinv = (1.0 / np.sqrt(slots.astype(np.float32))).astype(np.float32)
    xprime = (x[:, 0] * dinv).astype(np.float32)

    ptr = np.zeros(N + 1, np.int64)
    np.cumsum(indeg, out=ptr[1:])
    es = np.argsort(dst, kind="stable")
    sdst = dst[es]
    ssrc = src[es]
    rank = np.arange(E, dtype=np.int64) - ptr[sdst]

    # ---- layer 1: one section keyed by slots ----
    secA = _Section(slots)
    geomA = _plan_pieces([secA])
    GC_A = geomA[0]
    RPT_A = geomA[1]
    # per-node flat offsets into [N_CORES*128*GCOLS] / [N_CORES*128*RPT]
    gflatA = np.zeros(N, np.int64)
    oflatA = np.zeros(N, np.int64)
    gflatA[secA.nodes] = (secA.core * 128 + secA.p) * GC_A + secA.gbase
    oflatA[secA.nodes] = (secA.core * 128 + secA.p) * RPT_A + secA.obase

    GA = np.zeros(N_CORES * 128 * GC_A, bfloat16)
    xb = xprime.astype(bfloat16)
    GA[gflatA[sdst] + rank] = xb[ssrc]
    GA[gflatA + indeg] = xb                             # self slot (last)

    sumsA = _run(geomA, GA.reshape(N_CORES, 128, GC_A)).reshape(-1)
    y1p = (dinv * dinv * sumsA[oflatA]).astype(np.float32)   # y1' = d2*sum

    # ---- layer 2: sign-split sections ----
    y1b = y1p.astype(bfloat16)
    mB = y1b[ssrc]                                      # per-edge message
    q = (mB > 0)
    posb = np.bincount(sdst[q], minlength=N).astype(np.int64)
    self_pos = (y1b >= 0)                               # zeros -> P grid
    pslots = posb + self_pos
    mslots = (indeg - posb) + (~self_pos)
    excl = np.cumsum(q.astype(np.int64)) - q            # positives before e
    rank_pos = excl - excl[ptr[sdst]]                   # ...within segment
    rank_neg = rank - rank_pos

    secP = _Section(pslots)
    secM = _Section(mslots)
    geomB = _plan_pieces([secP, secM])
    GC_B = geomB[0]
    RPT_B = geomB[1]
    gflatP = np.zeros(N, np.int64)
    oflatP = np.zeros(N, np.int64)
    gflatP[secP.nodes] = (secP.core * 128 + secP.p) * GC_B + secP.gbase
    oflatP[secP.nodes] = (secP.core * 128 + secP.p) * RPT_B + secP.obase
    gflatM = np.zeros(N, np.int64)
    oflatM = np.zeros(N, np.int64)
    gflatM[secM.nodes] = ((secM.core * 128 + secM.p) * GC_B +
                          secM.gcols * 0 + secM.gbase + secP.gcols)
    oflatM[secM.nodes] = ((secM.core * 128 + secM.p) * RPT_B +
                          secM.obase + secP.rpt)

    GB = np.zeros(N_CORES * 128 * GC_B, bfloat16)
    fe = np.where(q, gflatP[sdst] + rank_pos, gflatM[sdst] + rank_neg)
    GB[fe] = mB
    fs = np.where(self_pos, gflatP + posb, gflatM + (indeg - posb))
    GB[fs] = y1b

    sumsB = _run(geomB, GB.reshape(N_CORES, 128, GC_B)).reshape(-1)
    sp = np.zeros(N, np.float32)
    sm = np.zeros(N, np.float32)
    sp[secP.nodes] = sumsB[oflatP[secP.nodes]]
    sm[secM.nodes] = sumsB[oflatM[secM.nodes]]

    # ---- O(N) host finalize ----
    aj = (np.maximum(W1, 0.0) @ W2).astype(np.float32)  # [4]
    cj = (np.minimum(W1, 0.0) @ W2).astype(np.float32)
    out = (dinv[:, None] *
           (sp[:, None] * aj[None, :] + sm[:, None] * cj[None, :]) +
           b2[None, :])
    return np.ascontiguousarray(out, dtype=np.float32)


# revision 13
# speedup vs baseline: 1.0308x; 1.0308x over previous
"""GCN (2-layer, PyG GCNConv semantics) on 8 Trainium2 NeuronCores.

Strategy (v3)
-------------
Host does layout + O(N) normalization; each NeuronCore does only dense
row-sum reductions over degree-class-padded bf16 message grids.

  layer-1 grid slot for edge u->v : x'(u)  = dinv(u)*x(u)   (+ self slot)
  layer-2 grids                   : y1'(u) = dinv(u)*y1(u)  (+ self slot)

With C_in == 1 and b1 == 0, layer 2 needs only s+(v) = sum of positive
incoming messages and s-(v) = sum of negative ones:
  out[v,j] = dinv_v*(a_j*s+ + c_j*s-) + b2_j,
  a_j = sum_{W1c>0} W1c*W2[c,j],  c_j = sum_{W1c<0} W1c*W2[c,j].
The host *sign-splits* the layer-2 messages into a positives grid and a
negatives grid (same total slot count!), so the device computes s+/s- as
plain row-sums - no abs/relu pass, no second reduction sweep.

Both NEFFs are instances of one generic "chunked grid row-sum" kernel:
DMA a column chunk, then per degree-class piece do an optional bf16
pairwise-add halving (DVE 2x mode) followed by a 1x tensor_reduce.
Node->row packing, unpacking, y1' = dinv^2*sums, and the 4-wide output
combine are all O(N)/O(E) index work + O(N) flops on the host.

Nodes are sorted by row width and dealt round-robin to the 8 cores
(independently per grid), so all cores share one SPMD NEFF geometry and
work is balanced to <0.1%; no collectives (a node's in-edges live on one
core).
"""
import sys

sys.path.insert(0, "/opt/trn_rl_repo")

import numpy as np

N_CORES = 8
CLS_STEP = 4          # slot-count class granularity
CHUNK_COLS = 4608     # <=9.2KB/partition bf16 per DMA chunk
HALVE_MIN_COLS = 768  # min piece cols to add the 2x pairwise-add pass
GPSIMD_HALVE_TOP = 2  # largest pieces whose halving runs on GpSimd
HALVE2_MIN_COLS = 3072  # min cols for a second halving level (needs S%8==0)

_NEFF_CACHE: dict = {}


class _Section:
    """Packing of one grid section (one slot-count distribution)."""

    def __init__(self, slot_counts):
        """slot_counts: [N] ints (0 => node absent from this section)."""
        n = slot_counts.shape[0]
        active = np.flatnonzero(slot_counts > 0)
        sc = slot_counts[active]
        order = np.argsort(sc, kind="stable")
        nodes = active[order]                   # width-sorted node ids
        widths = sc[order]
        clsS = (CLS_STEP * np.ceil(widths / CLS_STEP)).astype(np.int64)
        Svals, starts, cnts = np.unique(clsS, return_index=True,
                                        return_counts=True)
        self.classes = []                       # [(S, rpp)]
        for S, cnt in zip(Svals, cnts):
            npc = -(-int(cnt) // N_CORES)
            self.classes.append((int(S), -(-npc // 128)))
        # per active node: core, partition, row, class S
        i = np.arange(nodes.shape[0], dtype=np.int64)
        ci = np.searchsorted(Svals, clsS)
        rpp_arr = np.array([r for _, r in self.classes], np.int64)
        S_arr = Svals
        core = i % N_CORES
        q = (i - starts[ci]) // N_CORES
        p = q // rpp_arr[ci]
        r = q % rpp_arr[ci]
        goff = np.zeros(len(Svals), np.int64)
        ooff = np.zeros(len(Svals), np.int64)
        go = oo = 0
        for k, (S, rpp) in enumerate(self.classes):
            goff[k] = go
            ooff[k] = oo
            go += S * rpp
            oo += rpp
        self.gcols = go
        self.rpt = oo
        # per-active-node placement; caller adds section bases + core
        # stride to form flat indices.
        self.nodes = nodes
        self.core = core
        self.gbase = goff[ci] + r * S_arr[ci]
        self.obase = ooff[ci] + r
        self.p = p


def _plan_pieces(sections):
    """Lay out sections side by side in one [128, GCOLS] grid with one
    [128, RPT] sums output; return (GCOLS, RPT, chunks)."""
    pieces = []
    gbase = obase = 0
    for sec in sections:
        go = oo = 0
        for S, rpp in sec.classes:
            max_rows = max(1, CHUNK_COLS // S)
            r = 0
            while r < rpp:
                rows = min(max_rows, rpp - r)
                pieces.append((S, gbase + go + r * S, rows,
                               obase + oo + r))
                r += rows
            go += S * rpp
            oo += rpp
        gbase += sec.gcols
        obase += sec.rpt
    GCOLS, RPT = gbase, obase

    # pack pieces (grid-contiguous) into DMA chunks
    chunks = []
    cur, cur_cols = [], 0
    for (S, g0, rows, o0) in pieces:
        cols = rows * S
        if cur and cur_cols + cols > CHUNK_COLS:
            chunks.append(cur)
            cur, cur_cols = [], 0
        cur.append((S, g0, g0 + cols, o0, o0 + rows))
        cur_cols += cols
    if cur:
        chunks.append(cur)
    out = [(ch[0][1], ch[-1][2], ch) for ch in chunks]
    # ascending size: a small first chunk starts the DVE quickly; with
    # bufs >= n_chunks every chunk is in flight so DMA never stalls
    out.sort(key=lambda c: c[1] - c[0])
    return GCOLS, RPT, out


def _build_neff(geom):
    """Generic chunked row-sum kernel: sums[:, o] = rowsum(g[:, piece])."""
    from concourse import bacc, mybir, tile

    GCOLS, RPT, chunks = geom
    nc = bacc.Bacc("TRN2", target_bir_lowering=False, debug=False,
                   num_devices=N_CORES, enable_partition_id=False)
    f32, bf16 = mybir.dt.float32, mybir.dt.bfloat16
    add = mybir.AluOpType.add
    X = mybir.AxisListType.X
    g = nc.dram_tensor("g", [128, GCOLS], bf16, kind="ExternalInput")
    sm = nc.dram_tensor("sm", [128, RPT], f32, kind="ExternalOutput")

    all_pieces = [p for _, _, pcs in chunks for p in pcs]
    gp_pieces = set()
    for p in sorted(all_pieces, key=lambda p: p[1] - p[2])[:GPSIMD_HALVE_TOP]:
        if p[2] - p[1] >= HALVE_MIN_COLS and p[0] % 2 == 0:
            gp_pieces.add(p)

    with tile.TileContext(nc) as tc:
        with tc.tile_pool(name="p", bufs=max(2, len(chunks))) as pool, \
             tc.tile_pool(name="h", bufs=3) as hpool, \
             tc.tile_pool(name="s", bufs=1) as spool:
            sums = spool.tile([128, RPT], f32)
            for (g0, g1, pcs) in chunks:
                t = pool.tile([128, g1 - g0], bf16, tag="g")
                nc.sync.dma_start(out=t[:], in_=g.ap()[:, g0:g1])
                for (S, ig0, ig1, o0, o1) in pcs:
                    t3 = t[:, ig0 - g0:ig1 - g0].rearrange(
                        "p (r s) -> p r s", s=S)
                    if ig1 - ig0 >= HALVE_MIN_COLS and S % 2 == 0:
                        S2 = S // 2
                        eng = (nc.gpsimd if (S, ig0, ig1, o0, o1)
                               in gp_pieces else nc.vector)
                        h = hpool.tile([128, (o1 - o0) * S2], bf16,
                                       tag="h")
                        h3 = h[:].rearrange("p (r s) -> p r s", s=S2)
                        eng.tensor_tensor(
                            out=h3, in0=t3[:, :, 0:S2],
                            in1=t3[:, :, S2:S], op=add)
                        if (ig1 - ig0 >= HALVE2_MIN_COLS and S2 % 4 == 0
                                and eng is nc.vector):
                            S4 = S2 // 2
                            h2 = hpool.tile([128, (o1 - o0) * S4], bf16,
                                            tag="h2")
                            h23 = h2[:].rearrange("p (r s) -> p r s", s=S4)
                            nc.vector.tensor_tensor(
                                out=h23, in0=h3[:, :, 0:S4],
                                in1=h3[:, :, S4:S2], op=add)
                            nc.vector.tensor_reduce(
                                out=sums[:, o0:o1], in_=h23, axis=X,
                                op=add)
                        else:
                            nc.vector.tensor_reduce(
                                out=sums[:, o0:o1], in_=h3, axis=X,
                                op=add)
                    else:
                        nc.vector.tensor_reduce(
                            out=sums[:, o0:o1], in_=t3, axis=X, op=add)
            nc.sync.dma_start(out=sm.ap(), in_=sums[:])
    nc.compile()
    return nc


def _get_neff(geom_key, geom):
    if geom_key not in _NEFF_CACHE:
        _NEFF_CACHE[geom_key] = _build_neff(geom)
    return _NEFF_CACHE[geom_key]


def _geom_key(geom):
    GCOLS, RPT, chunks = geom
    return (GCOLS, RPT,
            tuple((g0, g1, tuple(pcs)) for g0, g1, pcs in chunks))


def _run(geom, grids):
    """grids: [N_CORES, 128, GCOLS] bf16 -> sums [N_CORES, 128, RPT]."""
    from concourse import bass_utils

    nc = _get_neff(_geom_key(geom), geom)
    in_maps = [{"g": grids[c]} for c in range(N_CORES)]
    res = bass_utils.run_bass_kernel_spmd(nc, in_maps,
                                          core_ids=list(range(N_CORES)))
    return np.stack([res.results[c]["sm"] for c in range(N_CORES)])


def kernel(x, edge_index, W1, b1, W2, b2):
    from ml_dtypes import bfloat16

    x = np.asarray(x, dtype=np.float32)
    W1 = np.asarray(W1, dtype=np.float32).reshape(-1)   # [4] (C_in == 1)
    b1 = np.asarray(b1, dtype=np.float32).reshape(-1)
    W2 = np.asarray(W2, dtype=np.float32)               # [4, 4]
    b2 = np.asarray(b2, dtype=np.float32).reshape(-1)
    ei = np.asarray(edge_index)
    N = x.shape[0]
    E = ei.shape[1]
    assert x.shape[1] == 1 and W1.shape[0] == 4 and W2.shape == (4, 4)
    # b1 == 0 is load-bearing for the s+/s- collapse (spec: fill zeros).
    assert np.all(b1 == 0.0), "kernel specialized to b1 == 0"

    src = ei[0].astype(np.int64)
    dst = ei[1].astype(np.int64)

    # ---- shared host index work ----
    indeg = np.bincount(dst, minlength=N).astype(np.int64)
    slots = indeg + 1                                   # + self slot
    dinv = (1.0 / np.sqrt(slots.astype(np.float32))).astype(np.float32)
    xprime = (x[:, 0] * dinv).astype(np.float32)

    ptr = np.zeros(N + 1, np.int64)
    np.cumsum(indeg, out=ptr[1:])
    es = np.argsort(dst, kind="stable")
    sdst = dst[es]
    ssrc = src[es]
    rank = np.arange(E, dtype=np.int64) - ptr[sdst]

    # ---- layer 1: one section keyed by slots ----
    secA = _Section(slots)
    geomA = _plan_pieces([secA])
    GC_A = geomA[0]
    RPT_A = geomA[1]
    # per-node flat offsets into [N_CORES*128*GCOLS] / [N_CORES*128*RPT]
    gflatA = np.zeros(N, np.int64)
    oflatA = np.zeros(N, np.int64)
    gflatA[secA.nodes] = (secA.core * 128 + secA.p) * GC_A + secA.gbase
    oflatA[secA.nodes] = (secA.core * 128 + secA.p) * RPT_A + secA.obase

    GA = np.zeros(N_CORES * 128 * GC_A, bfloat16)
    xb = xprime.astype(bfloat16)
    GA[gflatA[sdst] + rank] = xb[ssrc]
    GA[gflatA + indeg] = xb                             # self slot (last)

    sumsA = _run(geomA, GA.reshape(N_CORES, 128, GC_A)).reshape(-1)
    y1p = (dinv * dinv * sumsA[oflatA]).astype(np.float32)   # y1' = d2*sum

    # ---- layer 2: sign-split sections ----
    y1b = y1p.astype(bfloat16)
    mB = y1b[ssrc]                                      # per-edge message
    q = (mB > 0)
    posb = np.bincount(sdst[q], minlength=N).astype(np.int64)
    self_pos = (y1b >= 0)                               # zeros -> P grid
    pslots = posb + self_pos
    mslots = (indeg - posb) + (~self_pos)
    excl = np.cumsum(q.astype(np.int64)) - q            # positives before e
    rank_pos = excl - excl[ptr[sdst]]                   # ...within segment
    rank_neg = rank - rank_pos

    secP = _Section(pslots)
    secM = _Section(mslots)
    geomB = _plan_pieces([secP, secM])
    GC_B = geomB[0]
    RPT_B = geomB[1]
    gflatP = np.zeros(N, np.int64)
    oflatP = np.zeros(N, np.int64)
    gflatP[secP.nodes] = (secP.core * 128 + secP.p) * GC_B + secP.gbase
    oflatP[secP.nodes] = (secP.core * 128 + secP.p) * RPT_B + secP.obase
    gflatM = np.zeros(N, np.int64)
    oflatM = np.zeros(N, np.int64)
    gflatM[secM.nodes] = ((secM.core * 128 + secM.p) * GC_B +
                          secM.gcols * 0 + secM.gbase + secP.gcols)
    oflatM[secM.nodes] = ((secM.core * 128 + secM.p) * RPT_B +
                          secM.obase + secP.rpt)

    GB = np.zeros(N_CORES * 128 * GC_B, bfloat16)
    fe = np.where(q, gflatP[sdst] + rank_pos, gflatM[sdst] + rank_neg)
    GB[fe] = mB
    fs = np.where(self_pos, gflatP + posb, gflatM + (indeg - posb))
    GB[fs] = y1b

    sumsB = _run(geomB, GB.reshape(N_CORES, 128, GC_B)).reshape(-1)
    sp = np.zeros(N, np.float32)
    sm = np.zeros(N, np.float32)
    sp[secP.nodes] = sumsB[oflatP[secP.nodes]]
    sm[secM.nodes] = sumsB[oflatM[secM.nodes]]

    # ---- O(N) host finalize ----
    aj = (np.maximum(W1, 0.0) @ W2).astype(np.float32)  # [4]
    cj = (np.minimum(W1, 0.0) @ W2).astype(np.float32)
    out = (dinv[:, None] *
           (sp[:, None] * aj[None, :] + sm[:, None] * cj[None, :]) +
           b2[None, :])
    return np.ascontiguousarray(out, dtype=np.float32)


# revision 15
# speedup vs baseline: 1.0737x; 1.0416x over previous
"""GCN (2-layer, PyG GCNConv semantics) on 8 Trainium2 NeuronCores.

Strategy (v3)
-------------
Host does layout + O(N) normalization; each NeuronCore does only dense
row-sum reductions over degree-class-padded bf16 message grids.

  layer-1 grid slot for edge u->v : x'(u)  = dinv(u)*x(u)   (+ self slot)
  layer-2 grids                   : y1'(u) = dinv(u)*y1(u)  (+ self slot)

With C_in == 1 and b1 == 0, layer 2 needs only s+(v) = sum of positive
incoming messages and s-(v) = sum of negative ones:
  out[v,j] = dinv_v*(a_j*s+ + c_j*s-) + b2_j,
  a_j = sum_{W1c>0} W1c*W2[c,j],  c_j = sum_{W1c<0} W1c*W2[c,j].
The host *sign-splits* the layer-2 messages into a positives grid and a
negatives grid (same total slot count!), so the device computes s+/s- as
plain row-sums - no abs/relu pass, no second reduction sweep.

Both NEFFs are instances of one generic "chunked grid row-sum" kernel:
DMA a column chunk, then per degree-class piece do an optional bf16
pairwise-add halving (DVE 2x mode) followed by a 1x tensor_reduce.
Node->row packing, unpacking, y1' = dinv^2*sums, and the 4-wide output
combine are all O(N)/O(E) index work + O(N) flops on the host.

Nodes are sorted by row width and dealt round-robin to the 8 cores
(independently per grid), so all cores share one SPMD NEFF geometry and
work is balanced to <0.1%; no collectives (a node's in-edges live on one
core).
"""
import sys

sys.path.insert(0, "/opt/trn_rl_repo")

import numpy as np

N_CORES = 8
CLS_STEP = 4          # slot-count class granularity
CHUNK_COLS = 4608     # <=9.2KB/partition bf16 per DMA chunk
HALVE_MIN_COLS = 768  # min piece cols to add the 2x pairwise-add pass
GPSIMD_HALVE_TOP = 0  # gpsimd tensor_tensor measured ~4x slower than DVE 2x
FIRST_CHUNK_COLS = 384  # tiny lead chunk so the DVE starts ASAP
HALVE2_MIN_COLS = 3072  # min cols for a second halving level (needs S%8==0)

_NEFF_CACHE: dict = {}


class _Section:
    """Packing of one grid section (one slot-count distribution)."""

    def __init__(self, slot_counts):
        """slot_counts: [N] ints (0 => node absent from this section)."""
        n = slot_counts.shape[0]
        active = np.flatnonzero(slot_counts > 0)
        sc = slot_counts[active]
        order = np.argsort(sc, kind="stable")
        nodes = active[order]                   # width-sorted node ids
        widths = sc[order]
        clsS = (CLS_STEP * np.ceil(widths / CLS_STEP)).astype(np.int64)
        Svals, starts, cnts = np.unique(clsS, return_index=True,
                                        return_counts=True)
        self.classes = []                       # [(S, rpp)]
        for S, cnt in zip(Svals, cnts):
            npc = -(-int(cnt) // N_CORES)
            self.classes.append((int(S), -(-npc // 128)))
        # per active node: core, partition, row, class S
        i = np.arange(nodes.shape[0], dtype=np.int64)
        ci = np.searchsorted(Svals, clsS)
        rpp_arr = np.array([r for _, r in self.classes], np.int64)
        S_arr = Svals
        core = i % N_CORES
        q = (i - starts[ci]) // N_CORES
        p = q // rpp_arr[ci]
        r = q % rpp_arr[ci]
        goff = np.zeros(len(Svals), np.int64)
        ooff = np.zeros(len(Svals), np.int64)
        go = oo = 0
        for k, (S, rpp) in enumerate(self.classes):
            goff[k] = go
            ooff[k] = oo
            go += S * rpp
            oo += rpp
        self.gcols = go
        self.rpt = oo
        # per-active-node placement; caller adds section bases + core
        # stride to form flat indices.
        self.nodes = nodes
        self.core = core
        self.gbase = goff[ci] + r * S_arr[ci]
        self.obase = ooff[ci] + r
        self.p = p


def _plan_pieces(sections):
    """Lay out sections side by side in one [128, GCOLS] grid with one
    [128, RPT] sums output; return (GCOLS, RPT, chunks)."""
    pieces = []
    gbase = obase = 0
    for sec in sections:
        go = oo = 0
        for S, rpp in sec.classes:
            max_rows = max(1, CHUNK_COLS // S)
            r = 0
            while r < rpp:
                rows = min(max_rows, rpp - r)
                pieces.append((S, gbase + go + r * S, rows,
                               obase + oo + r))
                r += rows
            go += S * rpp
            oo += rpp
        gbase += sec.gcols
        obase += sec.rpt
    GCOLS, RPT = gbase, obase

    # pack pieces (grid-contiguous) into DMA chunks
    chunks = []
    cur, cur_cols = [], 0
    for (S, g0, rows, o0) in pieces:
        cols = rows * S
        if cur and cur_cols + cols > CHUNK_COLS:
            chunks.append(cur)
            cur, cur_cols = [], 0
        cur.append((S, g0, g0 + cols, o0, o0 + rows))
        cur_cols += cols
    if cur:
        chunks.append(cur)
    out = [(ch[0][1], ch[-1][2], ch) for ch in chunks]
    # ascending size: a small first chunk starts the DVE quickly; with
    # bufs >= n_chunks every chunk is in flight so DMA never stalls
    out.sort(key=lambda c: c[1] - c[0])
    # carve a tiny lead chunk so the first reduce isn't behind a big DMA
    g0, g1, pcs = out[0]
    if g1 - g0 > 2 * FIRST_CHUNK_COLS:
        lead, rest, acc = [], [], 0
        for (S, ig0, ig1, o0, o1) in pcs:
            if acc >= FIRST_CHUNK_COLS:
                rest.append((S, ig0, ig1, o0, o1))
                continue
            cols = ig1 - ig0
            if acc + cols > FIRST_CHUNK_COLS and cols > S:
                rows = max(1, (FIRST_CHUNK_COLS - acc) // S)
                rows = min(rows, (ig1 - ig0) // S - 1)
                mid_g = ig0 + rows * S
                lead.append((S, ig0, mid_g, o0, o0 + rows))
                rest.append((S, mid_g, ig1, o0 + rows, o1))
                acc += rows * S
            else:
                lead.append((S, ig0, ig1, o0, o1))
                acc += cols
        if lead and rest:
            out = [(lead[0][1], lead[-1][2], lead),
                   (rest[0][1], rest[-1][2], rest)] + out[1:]
    return GCOLS, RPT, out


def _build_neff(geom):
    """Generic chunked row-sum kernel: sums[:, o] = rowsum(g[:, piece])."""
    from concourse import bacc, mybir, tile

    GCOLS, RPT, chunks = geom
    nc = bacc.Bacc("TRN2", target_bir_lowering=False, debug=False,
                   num_devices=N_CORES, enable_partition_id=False)
    f32, bf16 = mybir.dt.float32, mybir.dt.bfloat16
    add = mybir.AluOpType.add
    X = mybir.AxisListType.X
    g = nc.dram_tensor("g", [128, GCOLS], bf16, kind="ExternalInput")
    sm = nc.dram_tensor("sm", [128, RPT], f32, kind="ExternalOutput")

    all_pieces = [p for _, _, pcs in chunks for p in pcs]
    gp_pieces = set()
    for p in sorted(all_pieces, key=lambda p: p[1] - p[2])[:GPSIMD_HALVE_TOP]:
        if p[2] - p[1] >= HALVE_MIN_COLS and p[0] % 2 == 0:
            gp_pieces.add(p)

    with tile.TileContext(nc) as tc:
        with tc.tile_pool(name="p", bufs=max(2, len(chunks))) as pool, \
             tc.tile_pool(name="h", bufs=3) as hpool, \
             tc.tile_pool(name="s", bufs=1) as spool:
            sums = spool.tile([128, RPT], f32)
            for (g0, g1, pcs) in chunks:
                t = pool.tile([128, g1 - g0], bf16, tag="g")
                nc.sync.dma_start(out=t[:], in_=g.ap()[:, g0:g1])
                for (S, ig0, ig1, o0, o1) in pcs:
                    t3 = t[:, ig0 - g0:ig1 - g0].rearrange(
                        "p (r s) -> p r s", s=S)
                    if ig1 - ig0 >= HALVE_MIN_COLS and S % 2 == 0:
                        S2 = S // 2
                        eng = (nc.gpsimd if (S, ig0, ig1, o0, o1)
                               in gp_pieces else nc.vector)
                        h = hpool.tile([128, (o1 - o0) * S2], bf16,
                                       tag="h")
                        h3 = h[:].rearrange("p (r s) -> p r s", s=S2)
                        eng.tensor_tensor(
                            out=h3, in0=t3[:, :, 0:S2],
                            in1=t3[:, :, S2:S], op=add)
                        if (ig1 - ig0 >= HALVE2_MIN_COLS and S2 % 4 == 0
                                and eng is nc.vector):
                            S4 = S2 // 2
                            h2 = hpool.tile([128, (o1 - o0) * S4], bf16,
                                            tag="h2")
                            h23 = h2[:].rearrange("p (r s) -> p r s", s=S4)
                            nc.vector.tensor_tensor(
                                out=h23, in0=h3[:, :, 0:S4],
                                in1=h3[:, :, S4:S2], op=add)
                            nc.vector.tensor_reduce(
                                out=sums[:, o0:o1], in_=h23, axis=X,
                                op=add)
                        else:
                            nc.vector.tensor_reduce(
                                out=sums[:, o0:o1], in_=h3, axis=X,
                                op=add)
                    else:
                        nc.vector.tensor_reduce(
                            out=sums[:, o0:o1], in_=t3, axis=X, op=add)
            nc.sync.dma_start(out=sm.ap(), in_=sums[:])
    nc.compile()
    return nc


def _get_neff(geom_key, geom):
    if geom_key not in _NEFF_CACHE:
        _NEFF_CACHE[geom_key] = _build_neff(geom)
    return _NEFF_CACHE[geom_key]


def _geom_key(geom):
    GCOLS, RPT, chunks = geom
    return (GCOLS, RPT,
            tuple((g0, g1, tuple(pcs)) for g0, g1, pcs in chunks))


def _run(geom, grids):
    """grids: [N_CORES, 128, GCOLS] bf16 -> sums [N_CORES, 128, RPT]."""
    from concourse import bass_utils

    nc = _get_neff(_geom_key(geom), geom)
    in_maps = [{"g": grids[c]} for c in range(N_CORES)]
    res = bass_utils.run_bass_kernel_spmd(nc, in_maps,
                                          core_ids=list(range(N_CORES)))
    return np.stack([res.results[c]["sm"] for c in range(N_CORES)])


def kernel(x, edge_index, W1, b1, W2, b2):
    from ml_dtypes import bfloat16

    x = np.asarray(x, dtype=np.float32)
    W1 = np.asarray(W1, dtype=np.float32).reshape(-1)   # [4] (C_in == 1)
    b1 = np.asarray(b1, dtype=np.float32).reshape(-1)
    W2 = np.asarray(W2, dtype=np.float32)               # [4, 4]
    b2 = np.asarray(b2, dtype=np.float32).reshape(-1)
    ei = np.asarray(edge_index)
    N = x.shape[0]
    E = ei.shape[1]
    assert x.shape[1] == 1 and W1.shape[0] == 4 and W2.shape == (4, 4)
    # b1 == 0 is load-bearing for the s+/s- collapse (spec: fill zeros).
    assert np.all(b1 == 0.0), "kernel specialized to b1 == 0"

    src = ei[0].astype(np.int64)
    dst = ei[1].astype(np.int64)

    # ---- shared host index work ----
    indeg = np.bincount(dst, minlength=N).astype(np.int64)
    slots = indeg + 1                                   # + self slot
    dinv = (1.0 / np.sqrt(slots.astype(np.float32))).astype(np.float32)
    xprime = (x[:, 0] * dinv).astype(np.float32)

    ptr = np.zeros(N + 1, np.int64)
    np.cumsum(indeg, out=ptr[1:])
    es = np.argsort(dst, kind="stable")
    sdst = dst[es]
    ssrc = src[es]
    rank = np.arange(E, dtype=np.int64) - ptr[sdst]

    # ---- layer 1: one section keyed by slots ----
    secA = _Section(slots)
    geomA = _plan_pieces([secA])
    GC_A = geomA[0]
    RPT_A = geomA[1]
    # per-node flat offsets into [N_CORES*128*GCOLS] / [N_CORES*128*RPT]
    gflatA = np.zeros(N, np.int64)
    oflatA = np.zeros(N, np.int64)
    gflatA[secA.nodes] = (secA.core * 128 + secA.p) * GC_A + secA.gbase
    oflatA[secA.nodes] = (secA.core * 128 + secA.p) * RPT_A + secA.obase

    GA = np.zeros(N_CORES * 128 * GC_A, bfloat16)
    xb = xprime.astype(bfloat16)
    GA[gflatA[sdst] + rank] = xb[ssrc]
    GA[gflatA + indeg] = xb                             # self slot (last)

    sumsA = _run(geomA, GA.reshape(N_CORES, 128, GC_A)).reshape(-1)
    y1p = (dinv * dinv * sumsA[oflatA]).astype(np.float32)   # y1' = d2*sum

    # ---- layer 2: sign-split sections ----
    y1b = y1p.astype(bfloat16)
    mB = y1b[ssrc]                                      # per-edge message
    q = (mB > 0)
    posb = np.bincount(sdst[q], minlength=N).astype(np.int64)
    self_pos = (y1b >= 0)                               # zeros -> P grid
    pslots = posb + self_pos
    mslots = (indeg - posb) + (~self_pos)
    excl = np.cumsum(q.astype(np.int64)) - q            # positives before e
    rank_pos = excl - excl[ptr[sdst]]                   # ...within segment
    rank_neg = rank - rank_pos

    secP = _Section(pslots)
    secM = _Section(mslots)
    geomB = _plan_pieces([secP, secM])
    GC_B = geomB[0]
    RPT_B = geomB[1]
    gflatP = np.zeros(N, np.int64)
    oflatP = np.zeros(N, np.int64)
    gflatP[secP.nodes] = (secP.core * 128 + secP.p) * GC_B + secP.gbase
    oflatP[secP.nodes] = (secP.core * 128 + secP.p) * RPT_B + secP.obase
    gflatM = np.zeros(N, np.int64)
    oflatM = np.zeros(N, np.int64)
    gflatM[secM.nodes] = ((secM.core * 128 + secM.p) * GC_B +
                          secM.gcols * 0 + secM.gbase + secP.gcols)
    oflatM[secM.nodes] = ((secM.core * 128 + secM.p) * RPT_B +
                          secM.obase + secP.rpt)

    GB = np.zeros(N_CORES * 128 * GC_B, bfloat16)
    fe = np.where(q, gflatP[sdst] + rank_pos, gflatM[sdst] + rank_neg)
    GB[fe] = mB
    fs = np.where(self_pos, gflatP + posb, gflatM + (indeg - posb))
    GB[fs] = y1b

    sumsB = _run(geomB, GB.reshape(N_CORES, 128, GC_B)).reshape(-1)
    sp = np.zeros(N, np.float32)
    sm = np.zeros(N, np.float32)
    sp[secP.nodes] = sumsB[oflatP[secP.nodes]]
    sm[secM.nodes] = sumsB[oflatM[secM.nodes]]

    # ---- O(N) host finalize ----
    aj = (np.maximum(W1, 0.0) @ W2).astype(np.float32)  # [4]
    cj = (np.minimum(W1, 0.0) @ W2).astype(np.float32)
    out = (dinv[:, None] *
           (sp[:, None] * aj[None, :] + sm[:, None] * cj[None, :]) +
           b2[None, :])
    return np.ascontiguousarray(out, dtype=np.float32)
